# revision 30
# baseline (speedup 1.0000x reference)
"""CreditRiskGNN (2-layer GCN) Trainium2 kernel, 8 NeuronCores.

Sharding: nodes sharded across the 8 cores; edges partitioned by destination
node so scatter-adds are core-local; per-shard node features all-gathered
between layers (bf16 to halve collective bytes).

Math: GCNConv(x, W, b)[d] = dinv[d] * (sum_{e: dst=d} h'[src_e] + h'[d]) + b
where h' = dinv (.) (x @ W) and dinv = rsqrt(indegree + 1) (self-loops).

Device pipeline per core (one NEFF, SPMD on all 8 cores; per-core data only):
  A) h'_shard = (dinv (.) x_shard) @ W1       (PE matmul)   -> f32 local +
     bf16 table slice
  B) AllGather bf16 h' -> full table [N, 64] bf16 in DRAM
  C) layer-1 aggregation per 128-dst tile: dma_gather of PAIRS of bf16 rows
     (256B descriptors, idx = src//2, SWDGE ucode on 4 queues), edges grouped
     by (tile, 65536-node window, src parity); one-hot dst-selection
     [kk, 128] built on DVE; PE matmuls accumulate in PSUM, rhs sliced to the
     parity half of each gathered pair; fused epilogue
     R' = dinv (.) relu(dinv (.) (agg + self) + b1)
  D) AllGather bf16 R'
  E) layer-2 aggregation over the same edge structure;
     y = sigmoid(dinv*(agg2@W2)+b2)

Within each 128-index gather run, indices are permuted so each SDMA engine's
8 partitions receive consecutive (sorted) source rows - better HBM locality.
Host does graph preprocessing only (index layout) and the final concat.
"""

import contextlib
import ctypes
import inspect
import math
import os
import sys
import textwrap
import types

import ml_dtypes
import numpy as np


def _patch_dma_gather_128():
    """Allow 128-byte gather elements (bass asserts a 256B minimum that the
    non-transpose ucode path does not require; the 256B stride-granularity
    constraint is still honored via elem_step)."""
    import concourse.bass as cbass

    cls = None
    for name in dir(cbass):
        obj = getattr(cbass, name)
        if isinstance(obj, type) and "dma_gather" in vars(obj):
            cls = obj
            break
    if cls is None:
        return
    fn = vars(cls)["dma_gather"]
    if getattr(fn, "_patched_128", False):
        return
    src = textwrap.dedent(inspect.getsource(fn))
    src = src.replace(
        "elem_size_bytes > 0 and elem_size_bytes % 256 == 0",
        "elem_size_bytes > 0 and elem_size_bytes % 128 == 0",
    )
    ns = {}
    exec(compile(src, "<dma_gather_128>", "exec"), vars(cbass), ns)
    new_fn = ns["dma_gather"]
    new_fn._patched_128 = True
    setattr(cls, "dma_gather", new_fn)

N_CORES = 8
P = 128
D_HID = 64
WINP = 32768               # pair-row window for int16 idx (= 65536 nodes)
MAX_IDX_PER_GATHER = 1024  # HW descriptor-ring limit (2048 hangs the queue)

LAST_RESULTS = None  # BassKernelResults of the last run (for test harnesses)


# ---------------------------------------------------------------------------
# axon NTFF profile hook shim (only needed when BASS_TRACE=1 under axon)
def _install_axon_profile_shim():
    if "antenv.axon_hooks" in sys.modules:
        return
    try:
        so_path = "/opt/axon/libaxon_pjrt.so"
        if not os.path.exists(so_path):
            return
        lib = ctypes.CDLL(so_path)
        if not hasattr(lib, "axon_start_nrt_profile"):
            return
        lib.axon_start_nrt_profile.argtypes = [
            ctypes.POINTER(ctypes.c_int64),
            ctypes.c_size_t,
        ]
        lib.axon_start_nrt_profile.restype = ctypes.c_int64
        lib.axon_stop_nrt_profile.argtypes = [ctypes.c_char_p]
        lib.axon_stop_nrt_profile.restype = ctypes.c_int64

        @contextlib.contextmanager
        def _hook(output_dir, device_ids):
            import jax

            jax.devices()
            if device_ids:
                ids = (ctypes.c_int64 * len(device_ids))(*device_ids)
                rc = lib.axon_start_nrt_profile(ids, len(device_ids))
            else:
                rc = lib.axon_start_nrt_profile(None, 0)
            if rc != 0:
                raise RuntimeError(f"axon_start_nrt_profile rc={rc}")
            try:
                yield
            finally:
                n = lib.axon_stop_nrt_profile(str(output_dir).encode())
                if n < 0:
                    raise RuntimeError(f"axon_stop_nrt_profile rc={n}")

        mod = types.ModuleType("antenv.axon_hooks")
        _state = {"hook": _hook}
        mod.set_axon_ntff_profile_hook = lambda h: _state.__setitem__("hook", h)
        mod.get_axon_ntff_profile_hook = lambda: _state["hook"]
        sys.modules["antenv.axon_hooks"] = mod
        import antenv

        antenv.axon_hooks = mod
    except Exception:
        pass


# ---------------------------------------------------------------------------
# Host-side graph preprocessing

# SDMA engine of an SBUF partition (port-swizzle): engine =
# ((p % 32) // 4) * 2 + (p // 64); permute each 128-idx run so one engine's 8
# partitions hold consecutive sorted sources (HBM read locality).
_ENG_OF_PART = np.array(
    [((p % 32) // 4) * 2 + (p // 64) for p in range(P)], dtype=np.int64
)
_ENGINE_PART_ORDER = np.argsort(_ENG_OF_PART, kind="stable").astype(np.int64)


def _wrap_idx_block(idxs_i16: np.ndarray) -> np.ndarray:
    """[n] int16 -> [128, n//16] in the SWDGE ucode layout: idx i at
    [i%16, i//16], replicated across the 8 groups of 16 partitions."""
    n = idxs_i16.shape[0]
    block = np.zeros((16, n // 16), dtype=np.int16)
    i = np.arange(n)
    block[i % 16, i // 16] = idxs_i16
    return np.tile(block, (8, 1))


def _build_plan(src, dst, n_nodes, n_cores):
    """Partition edges by destination shard; group per
    (dst-tile, pair-window, src-parity); pad each group to the max count
    across cores (rounded to 16) so the program shape is identical on every
    core."""
    sh = n_nodes // n_cores
    n_tiles = math.ceil(sh / P)
    n_pairs = (n_nodes + 1) // 2
    n_win = math.ceil(n_pairs / WINP)
    n_sub = n_win * 2  # (window, parity)

    core_of = dst // sh
    counts = np.zeros((n_cores, n_tiles, n_sub), dtype=np.int64)
    per_core_sorted = []
    for c in range(n_cores):
        m = core_of == c
        s_raw = src[m].astype(np.int64)
        d_c = (dst[m] - c * sh).astype(np.int64)
        # table rows live in half-major AllGather-chunk order:
        # node (co, r) -> hf*(n_cores*half) + co*half + (r - hf*half)
        half_rows = sh // 2
        assert sh % 4 == 0
        co_of = s_raw // sh
        r_of = s_raw % sh
        hfb = (r_of >= half_rows).astype(np.int64)
        s_c = hfb * (n_cores * half_rows) + co_of * half_rows + (
            r_of - hfb * half_rows
        )
        tile_id = d_c // P
        sub = ((s_c // 2) // WINP) * 2 + (s_c % 2)
        order = np.lexsort((s_c, sub, tile_id))
        s_c, d_c = s_c[order], d_c[order]
        key = tile_id[order] * n_sub + sub[order]
        allkeys = np.arange(n_tiles * n_sub)
        starts = np.searchsorted(key, allkeys, side="left").reshape(n_tiles, n_sub)
        ends = np.searchsorted(key, allkeys, side="right").reshape(n_tiles, n_sub)
        counts[c] = ends - starts
        per_core_sorted.append((s_c, d_c, starts))

    padded = counts.max(axis=0)
    padded = np.where(padded > 0, ((padded + 15) // 16) * 16, 0).astype(np.int64)
    # pad the even tile of each tile pair to a 128 multiple wherever its odd
    # sibling has the same (window, parity) subcell: lets one gather call span
    # both tiles with chunk columns staying 128-aligned at the boundary
    for tp in range(0, n_tiles - 1, 2):
        for s in range(n_sub):
            a, b = padded[tp, s], padded[tp + 1, s]
            if a > 0 and b > 0 and a + ((a + P - 1) // P) * P - a + b <= MAX_IDX_PER_GATHER:
                padded[tp, s] = ((a + P - 1) // P) * P

    # groups in gather (pair-interleaved) order: (pair, sub, tile)
    groups = []  # (t, sub, ni, idx_off16, gcol_off, dcol_off)
    tile_nch = np.zeros(n_tiles, dtype=np.int64)
    tile_chunk_off = np.zeros(n_tiles, dtype=np.int64)  # dl-column space
    # dl-column space: per-tile contiguous
    dcol = 0
    dcol_of = {}
    for t in range(n_tiles):
        tile_chunk_off[t] = dcol
        for s in range(n_sub):
            ni = int(padded[t, s])
            if ni == 0:
                continue
            dcol_of[(t, s)] = dcol
            dcol += (ni + P - 1) // P
        tile_nch[t] = dcol - tile_chunk_off[t]
    # gather space: pair-interleaved, one call per (pair, sub) where merged
    off16 = 0
    gcol = 0
    pair_calls = {}  # tp -> list of (sub, o16, ni_total, gcol0, ncg)
    for tp in range(0, n_tiles, 2):
        ts = [tp] + ([tp + 1] if tp + 1 < n_tiles else [])
        pair_calls[tp] = []
        for s in range(n_sub):
            nis = [int(padded[t, s]) for t in ts if padded[t, s] > 0]
            tls = [t for t in ts if padded[t, s] > 0]
            if not tls:
                continue
            tot = sum(nis)
            mergeable = (
                len(tls) == 2 and nis[0] % P == 0 and tot <= MAX_IDX_PER_GATHER
            )
            if mergeable:
                o16_0, g0 = off16, gcol
                for t, ni in zip(tls, nis):
                    groups.append((t, s, ni, off16, gcol, dcol_of[(t, s)]))
                    off16 += ni // 16
                    gcol += (ni + P - 1) // P
                pair_calls[tp].append((s, o16_0, tot, g0, gcol - g0))
            else:
                for t, ni in zip(tls, nis):
                    o16_0, g0 = off16, gcol
                    groups.append((t, s, ni, off16, gcol, dcol_of[(t, s)]))
                    off16 += ni // 16
                    gcol += (ni + P - 1) // P
                    pair_calls[tp].append((s, o16_0, ni, g0, gcol - g0))
    chunk_off = dcol

    chunk_k = np.zeros(int(chunk_off), dtype=np.int64)
    chunk_par = np.zeros(int(chunk_off), dtype=np.int64)
    gcol_of_chunk = np.zeros(int(chunk_off), dtype=np.int64)
    for (t, s, ni, o16, g0, d0) in groups:
        ncg = (ni + P - 1) // P
        for k in range(ncg):
            chunk_k[d0 + k] = min(P, ni - k * P)
            chunk_par[d0 + k] = s % 2
            gcol_of_chunk[d0 + k] = g0 + k

    meta = dict(
        n_nodes=n_nodes,
        n_pairs=n_pairs,
        sh=sh,
        n_tiles=n_tiles,
        n_win=n_win,
        groups=groups,
        pair_calls=pair_calls,
        tile_nch=tile_nch,
        tile_chunk_off=tile_chunk_off,
        chunk_k=chunk_k,
        chunk_par=chunk_par,
        gcol_of_chunk=gcol_of_chunk,
        total_idx=off16 * 16,
        total_chunks=int(chunk_off),
        total_gcols=int(gcol),
    )

    per_core = []
    for c in range(n_cores):
        s_c, d_c, starts = per_core_sorted[c]
        idx_arr = np.zeros((P, meta["total_idx"] // 16), dtype=np.int16)
        dst_arr = np.full((P, meta["total_chunks"]), -1.0, dtype=np.float32)
        for (t, s, ni, o16, g0, d0) in groups:
            ch_off = d0
            w = s // 2
            n_real = int(counts[c, t, s])
            st = int(starts[t, s])
            rel = np.zeros(ni, dtype=np.int64)
            dl = np.full(ni, -1.0, dtype=np.float32)
            if n_real > 0:
                rel[:n_real] = (s_c[st : st + n_real] // 2) - w * WINP
                dl[:n_real] = (d_c[st : st + n_real] % P).astype(np.float32)
            idx_arr[:, o16 : o16 + ni // 16] = _wrap_idx_block(
                rel.astype(np.int16)
            )
            i = np.arange(ni)
            dst_arr[i % P, ch_off + i // P] = dl
        per_core.append((idx_arr, dst_arr))
    return meta, per_core


# ---------------------------------------------------------------------------
# Device program


def _build_program(meta):
    import concourse.bacc as bacc
    import concourse.mybir as mybir
    import concourse.tile as tile

    _patch_dma_gather_128()

    n_nodes = meta["n_nodes"]
    n_pairs = meta["n_pairs"]
    sh = meta["sh"]
    n_tiles = meta["n_tiles"]
    pair_calls = meta["pair_calls"]
    tile_nch = meta["tile_nch"]
    tile_chunk_off = meta["tile_chunk_off"]
    chunk_k = meta["chunk_k"]
    chunk_par = meta["chunk_par"]
    gcol_of_chunk = meta["gcol_of_chunk"]
    total_idx = meta["total_idx"]
    total_chunks = meta["total_chunks"]

    f32 = mybir.dt.float32
    bf16 = mybir.dt.bfloat16
    nc = bacc.Bacc("TRN2", target_bir_lowering=False, debug=False, num_swdge_queues=4)

    xT = nc.dram_tensor("xT", [P, sh], bf16, kind="ExternalInput")
    w1 = nc.dram_tensor("w1", [P, D_HID], bf16, kind="ExternalInput")
    b1bc = nc.dram_tensor("b1bc", [P, D_HID], f32, kind="ExternalInput")
    w2bc = nc.dram_tensor("w2bc", [P, D_HID], f32, kind="ExternalInput")
    dinv_sh = nc.dram_tensor("dinv_sh", [P, n_tiles], f32, kind="ExternalInput")
    iota4 = nc.dram_tensor("iota4", [P, 4, P], bf16, kind="ExternalInput")
    idx16 = nc.dram_tensor(
        "idx16", [P, total_idx // 16], mybir.dt.int16, kind="ExternalInput"
    )
    dstloc = nc.dram_tensor("dstloc", [P, total_chunks], bf16, kind="ExternalInput")
    b2col = nc.dram_tensor("b2col", [P, 1], f32, kind="ExternalInput")
    y_out = nc.dram_tensor("y", [sh, 1], f32, kind="ExternalOutput")

    h_sh = nc.dram_tensor("h_sh", [sh, D_HID], f32, kind="Internal")
    r_sh = nc.dram_tensor("r_sh", [sh, D_HID], f32, kind="Internal")
    hb_sh = nc.dram_tensor("hb_sh", [sh, D_HID], bf16, kind="Internal")
    rb_sh = nc.dram_tensor("rb_sh", [sh, D_HID], bf16, kind="Internal")
    # full bf16 tables; +2 pad rows keep the odd-parity strided view in-bounds
    tbl_rows = 2 * n_pairs + 2
    hb_full = nc.dram_tensor(
        "hb_full", [tbl_rows, D_HID], bf16, kind="Internal", addr_space="Shared"
    )
    rb_full = nc.dram_tensor(
        "rb_full", [tbl_rows, D_HID], bf16, kind="Internal", addr_space="Shared"
    )

    rg = [list(range(N_CORES))]

    with tile.TileContext(nc) as tc:
        with (
            tc.tile_pool(name="const", bufs=1) as cpool,
            tc.tile_pool(name="sbuf", bufs=1) as pool,
            tc.tile_pool(name="psum", bufs=1, space="PSUM") as psum_pool,
        ):
            w1_t = cpool.tile([P, D_HID], bf16)
            nc.sync.dma_start(w1_t[:], w1[:])
            b1_t = cpool.tile([P, D_HID], f32)
            nc.sync.dma_start(b1_t[:], b1bc[:])
            w2_t = cpool.tile([P, D_HID], f32)
            nc.sync.dma_start(w2_t[:], w2bc[:])
            dinv_t = cpool.tile([P, n_tiles], f32)
            nc.sync.dma_start(dinv_t[:], dinv_sh[:])
            iota4_t = cpool.tile([P, 4, P], bf16)
            nc.sync.dma_start(iota4_t[:], iota4[:])

            # ---- phase A: h' = (dinv (.) x) @ W1 -> h_sh (f32) + hb_sh (bf16)
            B4 = 4
            for t4 in range(0, n_tiles, B4):
                nb = min(B4, n_tiles - t4)
                c0 = t4 * P
                cn = min(sh, (t4 + B4) * P) - c0
                xt = pool.tile([P, B4 * P], bf16, tag="xt", bufs=3)
                nc.sync.dma_start(xt[:, :cn], xT[:, c0 : c0 + cn])
                hs4 = pool.tile([P, B4, D_HID], f32, tag="hs", bufs=3)
                hb4 = pool.tile([P, B4, D_HID], bf16, tag="hb", bufs=3)
                for j in range(nb):
                    t = t4 + j
                    pt = min(P, sh - t * P)
                    ph = psum_pool.tile(
                        [P, D_HID], f32, tag="ph", bufs=2, space="PSUM"
                    )
                    nc.tensor.matmul(
                        ph[:pt, :],
                        lhsT=xt[:, j * P : j * P + pt],
                        rhs=w1_t[:],
                        start=True,
                        stop=True,
                    )
                    nc.vector.tensor_copy(out=hs4[:pt, j, :], in_=ph[:pt, :])
                    nc.scalar.copy(out=hb4[:pt, j, :], in_=ph[:pt, :])
                if cn == nb * P:
                    nc.sync.dma_start(
                        h_sh[c0 : c0 + cn, :].rearrange("(j p) d -> p j d", p=P),
                        hs4[:, :nb, :],
                    )
                    nc.sync.dma_start(
                        hb_sh[c0 : c0 + cn, :].rearrange("(j p) d -> p j d", p=P),
                        hb4[:, :nb, :],
                    )
                else:
                    for j in range(nb):
                        t = t4 + j
                        pt = min(P, sh - t * P)
                        nc.sync.dma_start(
                            h_sh[t * P : t * P + pt, :], hs4[:pt, j, :]
                        )
                        nc.sync.dma_start(
                            hb_sh[t * P : t * P + pt, :], hb4[:pt, j, :]
                        )

            # L1 metadata loads issued after phase A so xt loads go first
            idx_t = cpool.tile([P, total_idx // 16], mybir.dt.int16)
            nc.sync.dma_start(idx_t[:], idx16[:])
            dl_t = cpool.tile([P, total_chunks], bf16)
            nc.sync.dma_start(dl_t[:], dstloc[:])
            b2_t = cpool.tile([P, 1], f32)
            nc.sync.dma_start(b2_t[:], b2col[:])

            # ---- phase B: AllGather h' (bf16) in 2 contiguous chunks
            # (table rows are in half-major block order); chunk 0 overlaps
            # the tail of phase A
            half = sh // 2
            blk = N_CORES * half
            for hf in range(2):
                lo = hf * half
                nc.gpsimd.collective_compute(
                    "AllGather",
                    mybir.AluOpType.bypass,
                    replica_groups=rg,
                    ins=[hb_sh[lo : lo + half, :]],
                    outs=[hb_full[hf * blk : (hf + 1) * blk, :]],
                )

            qn_state = [0]

            def agg_layer(table, self_src, layer):
                for tp in range(0, n_tiles, 2):
                    pair_ts = [tp] + ([tp + 1] if tp + 1 < n_tiles else [])
                    g_lo = min(
                        int(gcol_of_chunk[int(tile_chunk_off[t])])
                        for t in pair_ts
                        if tile_nch[t] > 0
                    ) if any(tile_nch[t] > 0 for t in pair_ts) else 0
                    pair_ncg = sum(
                        int(ncg) for (_, _, _, _, ncg) in pair_calls[tp]
                    )
                    if pair_ncg > 0:
                        gbuf = pool.tile(
                            [P, pair_ncg, D_HID], bf16, tag=f"g{layer}", bufs=4
                        )
                        for (sub, o16, ni, g0, ncg) in pair_calls[tp]:
                            w = sub // 2
                            par = sub % 2
                            basep = w * WINP
                            spanp = min(WINP, n_pairs - basep)
                            rows_ap = table[
                                2 * basep + par : 2 * (basep + spanp) + par, :
                            ].rearrange("(p a) d -> p a d", a=2)[:, 0, :]
                            done = 0
                            while done < ni:
                                take = min(MAX_IDX_PER_GATHER, ni - done)
                                if ni - done > MAX_IDX_PER_GATHER:
                                    take = (take // P) * P
                                nc.gpsimd.dma_gather(
                                    gbuf[
                                        :,
                                        g0 - g_lo + done // P : g0
                                        - g_lo
                                        + done // P
                                        + (take + P - 1) // P,
                                        :,
                                    ],
                                    rows_ap,
                                    idx_t[
                                        :,
                                        o16 + done // 16 : o16 + (done + take) // 16,
                                    ],
                                    take,
                                    take,
                                    D_HID,
                                    elem_step=2 * D_HID,
                                    queue_num=qn_state[0] % 4,
                                )
                                qn_state[0] += 1
                                done += take
                    self._pair_state = (gbuf if pair_ncg > 0 else None, g_lo)
                    for t in pair_ts:
                        agg_tile(table, self_src, layer, t, *self._pair_state)

            class _NS:
                pass

            self = _NS()

            def agg_tile(table, self_src, layer, t, gbuf, g_lo):
                if True:
                    pt = min(P, sh - t * P)
                    nch = int(tile_nch[t])
                    ch0 = int(tile_chunk_off[t])
                    st = pool.tile([P, D_HID], f32, tag=f"st{layer}", bufs=4)
                    if pt < P:
                        nc.vector.memset(st[:], 0.0)
                    nc.sync.dma_start(st[:pt, :], self_src[t * P : t * P + pt, :])
                    if nch > 0:
                        pa = psum_pool.tile(
                            [P, D_HID], f32, tag=f"pa{layer}", bufs=3, space="PSUM"
                        )
                        for cb in range(0, nch, 4):
                            b = min(4, nch - cb)
                            oh = pool.tile(
                                [P, 4, P], bf16, tag=f"oh{layer}", bufs=6
                            )
                            dls = dl_t[:, ch0 + cb : ch0 + cb + b].rearrange(
                                "p (b o) -> p b o", o=1
                            )
                            nc.vector.tensor_tensor(
                                out=oh[:, :b, :],
                                in0=dls.to_broadcast([P, b, P]),
                                in1=iota4_t[:, :b, :],
                                op=mybir.AluOpType.is_equal,
                            )
                            for k in range(b):
                                ch = cb + k
                                kk = int(chunk_k[ch0 + ch])
                                gc = int(gcol_of_chunk[ch0 + ch]) - g_lo
                                nc.tensor.matmul(
                                    pa[:],
                                    lhsT=oh[:kk, k, :],
                                    rhs=gbuf[:kk, gc, :],
                                    start=(ch == 0),
                                    stop=(ch == nch - 1),
                                )
                    dv = dinv_t[:pt, t : t + 1]
                    if layer == 1:
                        t1 = pool.tile([P, D_HID], f32, tag="t1", bufs=4)
                        if nch > 0:
                            nc.vector.tensor_add(t1[:pt, :], pa[:pt, :], st[:pt, :])
                        else:
                            nc.vector.tensor_copy(out=t1[:pt, :], in_=st[:pt, :])
                        t2 = pool.tile([P, D_HID], f32, tag="t2", bufs=4)
                        nc.vector.tensor_tensor(
                            out=t2[:pt, :],
                            in0=t1[:pt, :],
                            in1=dv.to_broadcast([pt, D_HID]),
                            op=mybir.AluOpType.mult,
                        )
                        t3 = pool.tile([P, D_HID], f32, tag="t3", bufs=4)
                        nc.vector.tensor_add(t3[:pt, :], t2[:pt, :], b1_t[:pt, :])
                        rr = pool.tile([P, D_HID], f32, tag="rr", bufs=4)
                        nc.scalar.activation(
                            rr[:pt, :], t3[:pt, :], mybir.ActivationFunctionType.Relu
                        )
                        rp = pool.tile([P, D_HID], f32, tag="rp", bufs=4)
                        nc.vector.tensor_tensor(
                            out=rp[:pt, :],
                            in0=rr[:pt, :],
                            in1=dv.to_broadcast([pt, D_HID]),
                            op=mybir.AluOpType.mult,
                        )
                        rb = pool.tile([P, D_HID], bf16, tag="rb", bufs=4)
                        nc.scalar.copy(out=rb[:pt, :], in_=rp[:pt, :])
                        nc.sync.dma_start(r_sh[t * P : t * P + pt, :], rp[:pt, :])
                        nc.sync.dma_start(rb_sh[t * P : t * P + pt, :], rb[:pt, :])
                    else:
                        u1 = pool.tile([P, D_HID], f32, tag="u1", bufs=4)
                        if nch > 0:
                            nc.vector.tensor_add(u1[:pt, :], pa[:pt, :], st[:pt, :])
                        else:
                            nc.vector.tensor_copy(out=u1[:pt, :], in_=st[:pt, :])
                        u2 = pool.tile([P, D_HID], f32, tag="u2", bufs=4)
                        nc.vector.tensor_mul(u2[:pt, :], u1[:pt, :], w2_t[:pt, :])
                        yv = pool.tile([P, 1], f32, tag="yv", bufs=4)
                        nc.vector.tensor_reduce(
                            yv[:pt, :],
                            u2[:pt, :],
                            axis=mybir.AxisListType.X,
                            op=mybir.AluOpType.add,
                        )
                        ov = pool.tile([P, 1], f32, tag="ov", bufs=4)
                        nc.scalar.activation(
                            ov[:pt, :],
                            yv[:pt, :],
                            mybir.ActivationFunctionType.Sigmoid,
                            bias=b2_t[:pt, :],
                            scale=dv,
                        )
                        nc.sync.dma_start(y_out[t * P : t * P + pt, :], ov[:pt, :])

            # ---- phase C: layer 1 (table = hb_full, self rows = local h_sh)
            agg_layer(hb_full, h_sh, layer=1)

            # ---- phase D: AllGather R' (bf16), chunk 0 overlaps L1 tail
            for hf in range(2):
                lo = hf * half
                nc.gpsimd.collective_compute(
                    "AllGather",
                    mybir.AluOpType.bypass,
                    replica_groups=rg,
                    ins=[rb_sh[lo : lo + half, :]],
                    outs=[rb_full[hf * blk : (hf + 1) * blk, :]],
                )

            # ---- phase E: layer 2
            agg_layer(rb_full, r_sh, layer=2)

    nc.compile()
    return nc


# ---------------------------------------------------------------------------


def kernel(**inputs) -> np.ndarray:
    global LAST_RESULTS
    x = np.asarray(inputs["x"], dtype=np.float32)
    edge_index = np.asarray(inputs["edge_index"])
    w1_in = np.asarray(inputs["W1"], dtype=np.float32)
    b1_in = np.asarray(inputs["b1"], dtype=np.float32)
    w2_in = np.asarray(inputs["W2"], dtype=np.float32)
    b2_in = np.asarray(inputs["b2"], dtype=np.float32)

    n_nodes = x.shape[0]
    src = edge_index[0].astype(np.int64)
    dst = edge_index[1].astype(np.int64)

    deg = np.bincount(dst, minlength=n_nodes).astype(np.float64) + 1.0
    dinv = (1.0 / np.sqrt(deg)).astype(np.float32)

    meta, per_core = _build_plan(src, dst, n_nodes, N_CORES)
    sh = meta["sh"]
    n_tiles = meta["n_tiles"]

    nc = _build_program(meta)

    iota4_arr = (
        np.broadcast_to(np.arange(P, dtype=np.float32), (P, 4, P))
        .astype(ml_dtypes.bfloat16)
        .copy()
    )
    b1bc = np.broadcast_to(b1_in.reshape(1, D_HID), (P, D_HID)).copy()
    w2bc = np.broadcast_to(w2_in.reshape(1, D_HID), (P, D_HID)).copy()

    xsc = x * dinv[:, None]  # fold dinv into x: h' = (dinv.x) @ W1
    in_maps = []
    for c in range(N_CORES):
        idx_arr, dst_arr = per_core[c]
        xs = xsc[c * sh : (c + 1) * sh]  # [sh, 128]
        xT = np.ascontiguousarray(xs.T)  # [128, sh]
        dv = np.zeros((P, n_tiles), dtype=np.float32)
        dsl = dinv[c * sh : (c + 1) * sh]
        for t in range(n_tiles):
            pt = min(P, sh - t * P)
            dv[:pt, t] = dsl[t * P : t * P + pt]
        in_maps.append(
            {
                "xT": xT.astype(ml_dtypes.bfloat16),
                "w1": w1_in.astype(ml_dtypes.bfloat16),
                "b1bc": b1bc,
                "w2bc": w2bc,
                "dinv_sh": dv,
                "iota4": iota4_arr,
                "idx16": idx_arr,
                "dstloc": dst_arr.astype(ml_dtypes.bfloat16),
                "b2col": np.full((P, 1), float(b2_in.reshape(-1)[0]), dtype=np.float32),
            }
        )

    from concourse import bass_utils

    if os.environ.get("BASS_TRACE"):
        _install_axon_profile_shim()

    res = bass_utils.run_bass_kernel_spmd(
        nc,
        in_maps,
        core_ids=list(range(N_CORES)),
        trace=bool(os.environ.get("BASS_TRACE")),
        trace_cores=[0] if os.environ.get("BASS_TRACE") else None,
    )
    LAST_RESULTS = res
    out = np.concatenate([res.results[c]["y"] for c in range(N_CORES)], axis=0)
    return out.astype(np.float32)
